# revision 32
# baseline (speedup 1.0000x reference)
"""Trainium2 Bass kernel for nn_LocalRouter (sparse_attention).

Computation (reference semantics):
  local:  h_w = silu(mu_n @ Wm1_top + mu_{n-w} @ Wm1_bot + bm1), w=1..4
          local = mean_w(h_w) @ Wm2 + bm2
  global: scores = (mu @ Wq) @ (mu @ Wk)^T / sqrt(D), causal; top-8 -> softmax
          global = probs @ mu @ Wv + bv        (rows of probs sum to 1)
  out = concat([local, global]) @ Wo + bo

Algebraic refactors (host-side weight fusion, exact in fp32):
  scores = (mu @ Wqks) @ mu^T, Wqks = Wq @ Wk^T / sqrt(D)   [bk shifts a row
      uniformly -> no-op through top-k+softmax; bq term vanishes for bq==0]
  out = hbar @ Wmo + gsum @ Wvo + bconst
      hbar = sum_w silu(...), Wmo = (Wm2 @ Wo_top)/4, Wvo = Wv @ Wo_bot,
      gsum = sum_k p_k mu[idx_k], bconst = bo + bm2 @ Wo_top + bv @ Wo_bot

Sharding: core c -> batch b=c//2, half h=c%2 owns query tiles {t: t%2==h}
(interleaved for causal load balance).  Scores run in PE fp32: top-8
selection margins (8th vs 9th score) go down to 8e-7 on this data, so
reduced-precision score paths (bf16 3.6e-3, fp32r 1.5e-4 measured) flip
selections; one flip costs ~0.11 absmax vs the 0.015 gate.

v2: one resident muT copy ([d, 4+N] fp32) feeds qh/score/local matmuls as
strided slices -- no separate muq/muloc DRAM inputs.  SPMD-uniform program
uses h=1 geometry; h=0 cores get mu shifted right by 128 key-columns in
muT/mukeys (leading zeros), with a per-core head mask killing the zero block
in every strip.  Gather source is bf16.  Local branch is group-batched with
Silu on the scalar engine; bm1 folds into the A psum->sbuf copy.
"""

import math
import numpy as np
import ml_dtypes

B, N, D = 4, 4096, 512
WIN, TOPK = 4, 8
P = 128
NCORES = 8
NSLOT = 16            # query tiles owned per core
NEG = -1.0e30
NPAD = 4 + N          # muT free width incl. leading zero pad

_cache = {}


def _build_program():
    """Build the (core-uniform) Bass program once. Returns the compiled Bacc."""
    if "nc" in _cache:
        return _cache["nc"]
    from contextlib import ExitStack
    import concourse.bass as bass
    import concourse.tile as tile
    import concourse.mybir as mybir
    from concourse import bacc
    from concourse.masks import make_identity

    dt = mybir.dt
    AF = mybir.ActivationFunctionType
    OP = mybir.AluOpType

    nc = bacc.Bacc(
        "TRN2",
        target_bir_lowering=False,
        debug=False,
        enable_asserts=False,
        num_devices=NCORES,
    )

    f32, b16 = dt.float32, dt.bfloat16
    # ---- DRAM I/O (per-core data; program identical on all cores) ----
    muT = nc.dram_tensor("muT", [4, P, NPAD], f32, kind="ExternalInput").ap()
    muloc = nc.dram_tensor("muloc", [4, P, NSLOT * 132], b16,
                           kind="ExternalInput").ap()
    mukeys = nc.dram_tensor("mukeys", [N, D], b16, kind="ExternalInput").ap()
    wqks = nc.dram_tensor("wqks", [4, P, D], f32, kind="ExternalInput").ap()
    wm1t = nc.dram_tensor("wm1t", [4, P, D], b16, kind="ExternalInput").ap()
    wm1b = nc.dram_tensor("wm1b", [4, P, D], b16, kind="ExternalInput").ap()
    wmo = nc.dram_tensor("wmo", [4, P, D], b16, kind="ExternalInput").ap()
    wvo = nc.dram_tensor("wvo", [4, P, D], b16, kind="ExternalInput").ap()
    trimask = nc.dram_tensor("trimask", [P, P], f32, kind="ExternalInput").ap()
    headmask = nc.dram_tensor("headmask", [P, P], f32, kind="ExternalInput").ap()
    bm1t = nc.dram_tensor("bm1t", [P, 4], f32, kind="ExternalInput").ap()
    bconst = nc.dram_tensor("bconst", [P, 4], f32, kind="ExternalInput").ap()
    outT = nc.dram_tensor("outT", [4, P, NSLOT * P], f32, kind="ExternalOutput").ap()
    # scratch for the index-layout roundtrip (topk idx -> wrapped int16)
    iw_dram = nc.dram_tensor("iw_dram", [NSLOT, P, TOPK], dt.uint32,
                             kind="Internal").ap()

    with tile.TileContext(nc) as tc, ExitStack() as ctx:
        consts = ctx.enter_context(tc.tile_pool(name="consts", bufs=1))
        qh_pool = ctx.enter_context(tc.tile_pool(name="qh", bufs=2))
        strip_pool = ctx.enter_context(tc.tile_pool(name="strip", bufs=2))
        top_pool = ctx.enter_context(tc.tile_pool(name="top", bufs=3))
        g_pool = ctx.enter_context(tc.tile_pool(name="gather", bufs=3))
        acc_pool = ctx.enter_context(tc.tile_pool(name="acc", bufs=4))
        gt_pool = ctx.enter_context(tc.tile_pool(name="globT", bufs=2))
        loc_pool = ctx.enter_context(tc.tile_pool(name="loc", bufs=2))
        locw_pool = ctx.enter_context(tc.tile_pool(name="locw", bufs=1))
        muloc_pool = ctx.enter_context(tc.tile_pool(name="mulocp", bufs=1))
        hbar_pool = ctx.enter_context(tc.tile_pool(name="hbar", bufs=2))
        out_pool = ctx.enter_context(tc.tile_pool(name="outstage", bufs=1))

        ps_score = ctx.enter_context(tc.tile_pool(name="ps_score", bufs=2, space="PSUM"))
        ps_qh = ctx.enter_context(tc.tile_pool(name="ps_qh", bufs=1, space="PSUM"))
        ps_tp = ctx.enter_context(tc.tile_pool(name="ps_tp", bufs=1, space="PSUM"))
        ps_a = ctx.enter_context(tc.tile_pool(name="ps_a", bufs=1, space="PSUM"))
        ps_b = ctx.enter_context(tc.tile_pool(name="ps_b", bufs=1, space="PSUM"))
        ps_o = ctx.enter_context(tc.tile_pool(name="ps_o", bufs=1, space="PSUM"))

        # ---- resident constants ----
        # staged muT load: first 1056 cols (slots 0-1 scores + early qh)
        # land first so the PE can start ~25us earlier.
        muT_sb = consts.tile([P, 4, NPAD], f32)
        wqks_sb = consts.tile([P, 4, D], f32)
        for di in range(4):
            nc.sync.dma_start(wqks_sb[:, di, :], wqks[di])
        for di in range(4):
            nc.sync.dma_start(muT_sb[:, di, 0:1056], muT[di][:, 0:1056])
        for di in range(4):
            nc.sync.dma_start(muT_sb[:, di, 1056:], muT[di][:, 1056:])
        wm1t_sb = consts.tile([P, 4, D], b16)
        wm1b_sb = consts.tile([P, 4, D], b16)
        wmo_sb = consts.tile([P, 4, D], b16)
        wvo_sb = consts.tile([P, 4, D], b16)
        for sb, dr in ((wm1t_sb, wm1t), (wm1b_sb, wm1b),
                       (wmo_sb, wmo), (wvo_sb, wvo)):
            for di in range(4):
                nc.sync.dma_start(sb[:, di, :], dr[di])
        trimask_sb = consts.tile([P, P], f32)
        nc.sync.dma_start(trimask_sb[:], trimask[:])
        headmask_sb = consts.tile([P, P], f32)
        nc.sync.dma_start(headmask_sb[:], headmask[:])
        bm1t_sb = consts.tile([P, 4], f32)
        nc.sync.dma_start(bm1t_sb[:], bm1t[:])
        bconst_sb = consts.tile([P, 4], f32)
        nc.sync.dma_start(bconst_sb[:], bconst[:])
        ident16 = consts.tile([P, P], b16)
        make_identity(nc, ident16[:])

        slot_state = {}   # s -> dict(p8, g, globalT) front->back handoff
        loc_state = {}    # grp -> (a_sb, b_sb, hbar)

        def emit_front(s, globalT):
            """Query tile slot s (view tile t'=2s+1): scores (fp32) -> top8
            -> softmax -> gather ISSUE.  Gather-dependent consumption is in
            emit_back (one slot later) to keep engine FIFOs unblocked."""
            W = 256 * s + 256          # causal strip width, view geometry
            q0 = 4 + 256 * s + 128     # own-tile column start in muT view
            qh_ps = ps_qh.tile([P, 4, P], f32)
            for do in range(4):
                for di in range(4):
                    nc.tensor.matmul(
                        qh_ps[:, do, :],
                        wqks_sb[:, di, do * P:(do + 1) * P],
                        muT_sb[:, di, q0:q0 + P],
                        start=(di == 0), stop=(di == 3))
            qh = qh_pool.tile([P, 4, P], f32, tag="qh")
            with tc.high_priority(offset=120):
                nc.scalar.copy(qh[:], qh_ps[:])

            # scores strip [128 q, W keys] fp32; per-chunk top-8 candidates
            # (overlaps the max scan with remaining chunks' matmuls; only
            # find_index8 stays serial after the full strip)
            strip = strip_pool.tile([P, N], f32, tag="strip")
            cand = top_pool.tile([P, 64], f32, tag="cand")
            nchunks = (W + 511) // 512
            for c in range(nchunks):
                k0 = c * 512
                csz = min(512, W - k0)
                sps = ps_score.tile([P, 512], f32, tag="sps")
                for di in range(4):
                    nc.tensor.matmul(
                        sps[:, :csz],
                        qh[:, di, :],
                        muT_sb[:, di, 4 + k0:4 + k0 + csz],
                        start=(di == 0), stop=(di == 3))
                nc.scalar.copy(strip[:, k0:k0 + csz], sps[:, :csz])
                # masks must land before this chunk's max scan
                if k0 <= 0 < k0 + csz:
                    nc.vector.tensor_tensor(
                        strip[:, 0:P], strip[:, 0:P], headmask_sb[:],
                        op=OP.add)
                if c == nchunks - 1:
                    nc.vector.tensor_tensor(
                        strip[:, W - P:W], strip[:, W - P:W], trimask_sb[:],
                        op=OP.add)
                nc.vector.max(out=cand[:, 8 * c:8 * c + 8],
                              in_=strip[:, k0:k0 + csz])

            # top-8 values + indices
            v8 = top_pool.tile([P, TOPK], f32, tag="v8")
            nc.vector.max(out=v8[:], in_=cand[:, :8 * nchunks])
            i8 = top_pool.tile([P, TOPK], dt.uint32, tag="i8")
            nc.vector.max_index(out=i8[:], in_max=v8[:], in_values=strip[:, :W])
            # softmax over the 8 (masked entries are ~-1e30 -> exp ~ 0)
            nmax = top_pool.tile([P, 1], f32, tag="nmax")
            nc.vector.tensor_scalar_mul(nmax[:], v8[:, 0:1], -1.0)
            e8 = top_pool.tile([P, TOPK], f32, tag="e8")
            zsum = top_pool.tile([P, 1], f32, tag="zsum")
            nc.scalar.activation(e8[:], v8[:], AF.Exp, bias=nmax[:],
                                 accum_out=zsum[:])
            zr = top_pool.tile([P, 1], f32, tag="zr")
            nc.vector.reciprocal(zr[:], zsum[:])
            p8 = top_pool.tile([P, TOPK], f32, tag="p8")
            nc.vector.tensor_scalar_mul(p8[:], e8[:], zr[:])

            # gather index layout roundtrip (wrapped int16 for dma_gather):
            # wrapped[(k*128+p)%16, (k*128+p)//16] = i8[p, k], built with a
            # DRAM roundtrip + 8 replicate loads.  Issue the DMAs here; the
            # DVE cast + gather issue go in emit_front_b so wsum(s-1) can
            # run on the DVE during the DMA flight time.
            nc.sync.dma_start(iw_dram[s], i8[:])
            iw32 = top_pool.tile([P, TOPK * P // 16], dt.uint32, tag="iw32")
            flat = iw_dram[s].rearrange("a b -> (a b)")
            for rep in range(8):
                src_ap = bass.AP(flat.tensor, flat.offset,
                                 [[8, 16], [1, TOPK], [TOPK * 16, 8]])
                nc.sync.dma_start(
                    iw32[16 * rep:16 * rep + 16, :]
                    .rearrange("p (k h) -> p k h", k=TOPK), src_ap)
            slot_state[s] = (p8, iw32, globalT)

        def emit_front_b(s):
            """iw cast + gather issue for slot s (after back_dve(s-1))."""
            p8, iw32, globalT = slot_state.pop(s)
            iw = top_pool.tile([P, TOPK * P // 16], dt.int16, tag="iw")
            nc.vector.tensor_copy(iw[:], iw32[:])
            g = g_pool.tile([P, TOPK, D], b16, tag="g")
            nc.gpsimd.dma_gather(g[:], mukeys[:], iw[:], num_idxs=TOPK * P,
                                 num_idxs_reg=TOPK * P, elem_size=D)
            slot_state[s] = (p8, g, globalT)

        acc_state = {}

        def emit_back_dve(s):
            """Weighted sum for slot s (lag 1, emitted BEFORE the next front
            so it sits ahead of the strip-gated DVE chain in the FIFO)."""
            p8, g, globalT = slot_state.pop(s)
            # gsum[q, :] = sum_k p8[q,k] * g[q,k,:] (smallest weights first)
            acc = acc_pool.tile([P, D], b16, tag="acc")
            nc.vector.tensor_scalar_mul(acc[:], g[:, TOPK - 1, :],
                                        p8[:, TOPK - 1:TOPK])
            for k in range(TOPK - 2, -1, -1):
                nc.vector.scalar_tensor_tensor(
                    acc[:], g[:, k, :], p8[:, k:k + 1], acc[:],
                    op0=OP.mult, op1=OP.add)
            acc_state[s] = (acc, globalT)

        def emit_back_pe(s):
            """Transpose gsum -> globalT for slot s (lag 4: acc is ready well
            before the PE reaches these, so the PE FIFO rarely waits)."""
            acc, globalT = acc_state.pop(s)
            for j in range(4):
                tp = ps_tp.tile([P, P], b16, tag="tp")
                nc.tensor.transpose(tp[:], acc[:, j * P:(j + 1) * P], ident16[:])
                nc.vector.tensor_copy(
                    globalT[:, j, (s % 4) * P:(s % 4 + 1) * P], tp[:])

        def emit_local_mm(grp, hbar):
            """Local branch for own-tile group grp (4 own tiles, 512 rows):
            hbar = sum_w silu(A + B_shift(w) + bm1), group-batched.
            muloc strips: per own tile, bf16 rows [128t-4, 128t+128) as 132
            cols (zero-padded at n<0)."""
            muloc_g = loc_pref.pop(grp)
            a_sb = loc_pool.tile([P, 4, D], b16, tag="a_sb")
            b_sb = loc_pool.tile([P, 4, 2, 264], b16, tag="b_sb")
            for dh in range(4):
                a_ps = ps_a.tile([P, D], f32, tag="a_ps")
                for di in range(4):
                    mv = muloc_g[:, di, :] \
                        .rearrange("p (t c) -> p t c", c=132)[:, :, 4:132]
                    nc.tensor.matmul(
                        a_ps[:].rearrange("p (t c) -> p t c", c=128),
                        wm1t_sb[:, di, dh * P:(dh + 1) * P],
                        mv, start=(di == 0), stop=(di == 3))
                nc.scalar.activation(a_sb[:, dh, :], a_ps[:], AF.Identity,
                                     bias=bm1t_sb[:, dh:dh + 1])
                for half in range(2):
                    b_ps = ps_b.tile([P, 2, 132], f32, tag=f"b_ps{half}")
                    for di in range(4):
                        mv = muloc_g[:, di, 264 * half:264 * half + 264] \
                            .rearrange("p (t c) -> p t c", c=132)
                        nc.tensor.matmul(
                            b_ps[:], wm1b_sb[:, di, dh * P:(dh + 1) * P],
                            mv, start=(di == 0), stop=(di == 3))
                    nc.scalar.copy(
                        b_sb[:, dh, half, :].rearrange("p (t c) -> p t c", c=132),
                        b_ps[:])
            loc_state[grp] = (a_sb, b_sb, hbar)

        def emit_local_w(grp, w):
            """One shift w of the local branch: x = A(+bm1) + B_shift(w),
            hbar (+)= silu(x).  Emitted interleaved with the next group's
            slots to spread ACT/DVE load."""
            a_sb, b_sb, hbar = loc_state[grp]
            x = locw_pool.tile([P, 4, D], b16, tag="x")
            for dh in range(4):
                in0 = a_sb[:, dh, :].rearrange(
                    "p (a t c) -> p a t c", a=2, c=128)
                in1 = b_sb[:, dh].rearrange(
                    "p a (t c) -> p a t c", c=132)[:, :, :, 4 - w:132 - w]
                outw = x[:, dh, :].rearrange(
                    "p (a t c) -> p a t c", a=2, c=128)
                nc.vector.tensor_tensor(outw, in0, in1, op=OP.add)
            if w == 1:
                nc.scalar.activation(
                    hbar[:].rearrange("p a c -> p (a c)"),
                    x[:].rearrange("p a c -> p (a c)"), AF.Silu)
            else:
                sil = locw_pool.tile([P, 4, D], b16, tag="sil")
                nc.scalar.activation(
                    sil[:].rearrange("p a c -> p (a c)"),
                    x[:].rearrange("p a c -> p (a c)"), AF.Silu)
                nc.vector.tensor_tensor(
                    hbar[:].rearrange("p a c -> p (a c)"),
                    hbar[:].rearrange("p a c -> p (a c)"),
                    sil[:].rearrange("p a c -> p (a c)"), op=OP.add)

        def emit_outproj(grp):
            _, _, hbar = loc_state[grp]
            globalT = gt_done.pop(grp)
            r0 = grp * 512
            for do in range(4):
                o_ps = ps_o.tile([P, 512], f32, tag="o_ps")
                for dm in range(4):
                    nc.tensor.matmul(
                        o_ps[:],
                        wmo_sb[:, dm, do * P:(do + 1) * P],
                        hbar[:, dm, :],
                        start=(dm == 0), stop=False)
                for dm in range(4):
                    nc.tensor.matmul(
                        o_ps[:],
                        wvo_sb[:, dm, do * P:(do + 1) * P],
                        globalT[:, dm, :],
                        start=False, stop=(dm == 3))
                ost = out_pool.tile([P, 512], f32, tag="ost")
                nc.scalar.activation(ost[:], o_ps[:], AF.Identity,
                                     bias=bconst_sb[:, do:do + 1])
                nc.sync.dma_start(outT[do, :, r0:r0 + 512], ost[:])

        gt_done = {}
        loc_pref = {}

        def emit_local_pref(grp):
            muloc_g = muloc_pool.tile([P, 4, 528], b16, tag="mulocg")
            for di in range(4):
                nc.sync.dma_start(muloc_g[:, di, :],
                                  muloc[di][:, 528 * grp:528 * grp + 528])
            loc_pref[grp] = muloc_g
        # Software-pipelined emission: slot fronts run one ahead of backs;
        # local silu work and outproj lag a full group so nothing on the
        # PE/ACT FIFOs ever waits on a gather chain.
        for grp in range(4):
            globalT = gt_pool.tile([P, 4, 512], b16, tag="globalT")
            gt_done[grp] = globalT
            hbar = hbar_pool.tile([P, 4, 512], b16, tag="hbar")
            emit_local_pref(grp)
            for j, s in enumerate(range(4 * grp, 4 * grp + 4)):
                emit_front(s, globalT)
                if s > 0:
                    with tc.high_priority(offset=400):
                        emit_back_dve(s - 1)
                emit_front_b(s)
                if s > 3:
                    with tc.high_priority(offset=150):
                        emit_back_pe(s - 4)
                if grp > 0:
                    for w in ([1, 2] if j == 0 else [3, 4] if j == 1 else []):
                        emit_local_w(grp - 1, w)
            if grp > 0:
                emit_outproj(grp - 1)
            emit_local_mm(grp, hbar)
        emit_back_dve(NSLOT - 1)
        for s in range(NSLOT - 4, NSLOT):
            emit_back_pe(s)
        for w in range(1, WIN + 1):
            emit_local_w(3, w)
        emit_outproj(3)

    nc.compile()
    _cache["nc"] = nc
    return nc


def prep_in_maps(inputs):
    f32 = np.float32
    b16 = ml_dtypes.bfloat16
    mu = np.asarray(inputs["mu"], f32)
    Wq = np.asarray(inputs["Wq"], f32)
    bq = np.asarray(inputs["bq"], f32)
    Wk = np.asarray(inputs["Wk"], f32)
    Wv = np.asarray(inputs["Wv"], f32)
    bv = np.asarray(inputs["bv"], f32)
    Wm1 = np.asarray(inputs["Wm1"], f32)
    bm1 = np.asarray(inputs["bm1"], f32)
    Wm2 = np.asarray(inputs["Wm2"], f32)
    bm2 = np.asarray(inputs["bm2"], f32)
    Wo = np.asarray(inputs["Wo"], f32)
    bo = np.asarray(inputs["bo"], f32)
    assert not bq.any(), "bq != 0 unsupported (adds a per-key score term)"

    Wqks = (Wq @ Wk.T / math.sqrt(D)).astype(f32)
    Wmo = ((Wm2 @ Wo[:D]) / WIN).astype(f32)
    Wvo = (Wv @ Wo[D:]).astype(f32)
    bconst = (bo + bm2 @ Wo[:D] + bv @ Wo[D:]).astype(f32)
    consts = dict(
        wqks=np.ascontiguousarray(Wqks.reshape(4, P, D)),
        wm1t=np.ascontiguousarray(Wm1[:D]).reshape(4, P, D).astype(b16),
        wm1b=np.ascontiguousarray(Wm1[D:]).reshape(4, P, D).astype(b16),
        wmo=Wmo.reshape(4, P, D).astype(b16),
        wvo=Wvo.reshape(4, P, D).astype(b16),
        bm1t=np.ascontiguousarray(bm1.reshape(4, P).T),
        bconst=np.ascontiguousarray(bconst.reshape(4, P).T),
    )

    j = np.arange(P)[None, :]
    p = np.arange(P)[:, None]
    tril0 = np.where(j <= p, 0.0, NEG).astype(f32)

    in_maps = []
    for c in range(NCORES):
        b, h = c // 2, c % 2
        mub = np.ascontiguousarray(mu[b])                   # [N, D] f32
        muT_pad = np.zeros((4, P, NPAD), f32)
        keys = np.zeros((N, D), f32)
        if h == 1:
            muT_pad[:, :, 4:] = mub.T.reshape(4, P, N)
            keys[:] = mub
            hm = np.zeros((P, P), f32)
        else:
            # shift mu right by 128 key-columns: view[n'] = mu[n'-128]
            muT_pad[:, :, 4 + P:] = mub[:N - P].T.reshape(4, P, N - P)
            keys[P:] = mub[:N - P]
            hm = np.full((P, P), NEG, f32)
        # local strips: per own tile t, rows [128t-4, 128t+128) zero-padded
        strips = []
        for t in range(h, 32, 2):
            st = np.zeros((132, D), f32)
            lo = 128 * t - 4
            src_lo = max(lo, 0)
            st[src_lo - lo:] = mub[src_lo:128 * t + 128]
            strips.append(st)
        muloc = np.concatenate(strips)                      # [2112, D]
        muloc = np.ascontiguousarray(muloc.T).reshape(4, P, NSLOT * 132)
        in_maps.append(dict(
            muT=muT_pad,
            muloc=muloc.astype(b16),
            mukeys=keys.astype(b16),
            trimask=tril0,
            headmask=hm,
            **consts,
        ))
    return in_maps


def assemble(core_outs):
    """core_outs: list of outT arrays [4, P, 2048] per core -> full [B, N, D]."""
    out = np.empty((B, N, D), np.float32)
    for c in range(NCORES):
        b, h = c // 2, c % 2
        oT = np.asarray(core_outs[c])
        oc = np.ascontiguousarray(oT.reshape(D, NSLOT * P).T)  # [2048, D]
        for s, t in enumerate(range(h, 32, 2)):
            out[b, 128 * t:128 * t + 128] = oc[128 * s:128 * s + 128]
    return out


def kernel(**inputs):
    nc = _build_program()
    in_maps = prep_in_maps(inputs)

    import os
    from concourse.bass_utils import run_bass_kernel_spmd
    trace = bool(int(os.environ.get("LR_TRACE", "0")))
    res = run_bass_kernel_spmd(nc, in_maps, core_ids=list(range(NCORES)),
                               trace=trace)
    _cache["last_results"] = res
    return assemble([res.results[c]["outT"] for c in range(NCORES)])


# revision 33
# speedup vs baseline: 1.0025x; 1.0025x over previous
"""Trainium2 Bass kernel for nn_LocalRouter (sparse_attention).

Computation (reference semantics):
  local:  h_w = silu(mu_n @ Wm1_top + mu_{n-w} @ Wm1_bot + bm1), w=1..4
          local = mean_w(h_w) @ Wm2 + bm2
  global: scores = (mu @ Wq) @ (mu @ Wk)^T / sqrt(D), causal; top-8 -> softmax
          global = probs @ mu @ Wv + bv        (rows of probs sum to 1)
  out = concat([local, global]) @ Wo + bo

Algebraic refactors (host-side weight fusion, exact in fp32):
  scores = (mu @ Wqks) @ mu^T, Wqks = Wq @ Wk^T / sqrt(D)   [bk shifts a row
      uniformly -> no-op through top-k+softmax; bq term vanishes for bq==0]
  out = hbar @ Wmo + gsum @ Wvo + bconst
      hbar = sum_w silu(...), Wmo = (Wm2 @ Wo_top)/4, Wvo = Wv @ Wo_bot,
      gsum = sum_k p_k mu[idx_k], bconst = bo + bm2 @ Wo_top + bv @ Wo_bot

Sharding: core c -> batch b=c//2, half h=c%2 owns query tiles {t: t%2==h}
(interleaved for causal load balance).  Scores run in PE fp32: top-8
selection margins (8th vs 9th score) go down to 8e-7 on this data, so
reduced-precision score paths (bf16 3.6e-3, fp32r 1.5e-4 measured) flip
selections; one flip costs ~0.11 absmax vs the 0.015 gate.

v2: one resident muT copy ([d, 4+N] fp32) feeds qh/score/local matmuls as
strided slices -- no separate muq/muloc DRAM inputs.  SPMD-uniform program
uses h=1 geometry; h=0 cores get mu shifted right by 128 key-columns in
muT/mukeys (leading zeros), with a per-core head mask killing the zero block
in every strip.  Gather source is bf16.  Local branch is group-batched with
Silu on the scalar engine; bm1 folds into the A psum->sbuf copy.
"""

import math
import numpy as np
import ml_dtypes

B, N, D = 4, 4096, 512
WIN, TOPK = 4, 8
P = 128
NCORES = 8
NSLOT = 16            # query tiles owned per core
NEG = -1.0e30
NPAD = 4 + N          # muT free width incl. leading zero pad

_cache = {}


def _build_program():
    """Build the (core-uniform) Bass program once. Returns the compiled Bacc."""
    if "nc" in _cache:
        return _cache["nc"]
    from contextlib import ExitStack
    import concourse.bass as bass
    import concourse.tile as tile
    import concourse.mybir as mybir
    from concourse import bacc
    from concourse.masks import make_identity

    dt = mybir.dt
    AF = mybir.ActivationFunctionType
    OP = mybir.AluOpType

    nc = bacc.Bacc(
        "TRN2",
        target_bir_lowering=False,
        debug=False,
        enable_asserts=False,
        num_devices=NCORES,
    )

    f32, b16 = dt.float32, dt.bfloat16
    # ---- DRAM I/O (per-core data; program identical on all cores) ----
    muT = nc.dram_tensor("muT", [4, P, NPAD], f32, kind="ExternalInput").ap()
    muloc = nc.dram_tensor("muloc", [4, P, NSLOT * 132], b16,
                           kind="ExternalInput").ap()
    mukeys = nc.dram_tensor("mukeys", [N, D], b16, kind="ExternalInput").ap()
    wqks = nc.dram_tensor("wqks", [4, P, D], f32, kind="ExternalInput").ap()
    wm1t = nc.dram_tensor("wm1t", [4, P, D], b16, kind="ExternalInput").ap()
    wm1b = nc.dram_tensor("wm1b", [4, P, D], b16, kind="ExternalInput").ap()
    wmo = nc.dram_tensor("wmo", [4, P, D], b16, kind="ExternalInput").ap()
    wvo = nc.dram_tensor("wvo", [4, P, D], b16, kind="ExternalInput").ap()
    trimask = nc.dram_tensor("trimask", [P, P], f32, kind="ExternalInput").ap()
    headmask = nc.dram_tensor("headmask", [P, P], f32, kind="ExternalInput").ap()
    bm1t = nc.dram_tensor("bm1t", [P, 4], f32, kind="ExternalInput").ap()
    bconst = nc.dram_tensor("bconst", [P, 4], f32, kind="ExternalInput").ap()
    outT = nc.dram_tensor("outT", [4, P, NSLOT * P], f32, kind="ExternalOutput").ap()
    # scratch for the index-layout roundtrip (topk idx -> wrapped int16)
    iw_dram = nc.dram_tensor("iw_dram", [NSLOT, P, TOPK], dt.uint32,
                             kind="Internal").ap()

    with tile.TileContext(nc) as tc, ExitStack() as ctx:
        consts = ctx.enter_context(tc.tile_pool(name="consts", bufs=1))
        qh_pool = ctx.enter_context(tc.tile_pool(name="qh", bufs=2))
        strip_pool = ctx.enter_context(tc.tile_pool(name="strip", bufs=2))
        top_pool = ctx.enter_context(tc.tile_pool(name="top", bufs=3))
        g_pool = ctx.enter_context(tc.tile_pool(name="gather", bufs=3))
        acc_pool = ctx.enter_context(tc.tile_pool(name="acc", bufs=4))
        gt_pool = ctx.enter_context(tc.tile_pool(name="globT", bufs=2))
        loc_pool = ctx.enter_context(tc.tile_pool(name="loc", bufs=2))
        locw_pool = ctx.enter_context(tc.tile_pool(name="locw", bufs=1))
        muloc_pool = ctx.enter_context(tc.tile_pool(name="mulocp", bufs=1))
        hbar_pool = ctx.enter_context(tc.tile_pool(name="hbar", bufs=2))
        out_pool = ctx.enter_context(tc.tile_pool(name="outstage", bufs=1))

        ps_score = ctx.enter_context(tc.tile_pool(name="ps_score", bufs=2, space="PSUM"))
        ps_qh = ctx.enter_context(tc.tile_pool(name="ps_qh", bufs=1, space="PSUM"))
        ps_tp = ctx.enter_context(tc.tile_pool(name="ps_tp", bufs=1, space="PSUM"))
        ps_a = ctx.enter_context(tc.tile_pool(name="ps_a", bufs=1, space="PSUM"))
        ps_b = ctx.enter_context(tc.tile_pool(name="ps_b", bufs=1, space="PSUM"))
        ps_o = ctx.enter_context(tc.tile_pool(name="ps_o", bufs=1, space="PSUM"))

        # ---- resident constants ----
        # staged muT load: first 1056 cols (slots 0-1 scores + early qh)
        # land first so the PE can start ~25us earlier.
        muT_sb = consts.tile([P, 4, NPAD], f32)
        wqks_sb = consts.tile([P, 4, D], f32)
        for di in range(4):
            nc.sync.dma_start(wqks_sb[:, di, :], wqks[di])
        for di in range(4):
            nc.sync.dma_start(muT_sb[:, di, 0:1056], muT[di][:, 0:1056])
        for di in range(4):
            nc.sync.dma_start(muT_sb[:, di, 1056:], muT[di][:, 1056:])
        wm1t_sb = consts.tile([P, 4, D], b16)
        wm1b_sb = consts.tile([P, 4, D], b16)
        wmo_sb = consts.tile([P, 4, D], b16)
        wvo_sb = consts.tile([P, 4, D], b16)
        for sb, dr in ((wm1t_sb, wm1t), (wm1b_sb, wm1b),
                       (wmo_sb, wmo), (wvo_sb, wvo)):
            for di in range(4):
                nc.sync.dma_start(sb[:, di, :], dr[di])
        trimask_sb = consts.tile([P, P], f32)
        nc.sync.dma_start(trimask_sb[:], trimask[:])
        headmask_sb = consts.tile([P, P], f32)
        nc.sync.dma_start(headmask_sb[:], headmask[:])
        bm1t_sb = consts.tile([P, 4], f32)
        nc.sync.dma_start(bm1t_sb[:], bm1t[:])
        bconst_sb = consts.tile([P, 4], f32)
        nc.sync.dma_start(bconst_sb[:], bconst[:])
        ident16 = consts.tile([P, P], b16)
        make_identity(nc, ident16[:])

        slot_state = {}   # s -> dict(p8, g, globalT) front->back handoff
        loc_state = {}    # grp -> (a_sb, b_sb, hbar)

        def emit_front(s, globalT):
            """Query tile slot s (view tile t'=2s+1): scores (fp32) -> top8
            -> softmax -> gather ISSUE.  Gather-dependent consumption is in
            emit_back (one slot later) to keep engine FIFOs unblocked."""
            W = 256 * s + 256          # causal strip width, view geometry
            q0 = 4 + 256 * s + 128     # own-tile column start in muT view
            qh_ps = ps_qh.tile([P, 4, P], f32)
            for do in range(4):
                for di in range(4):
                    nc.tensor.matmul(
                        qh_ps[:, do, :],
                        wqks_sb[:, di, do * P:(do + 1) * P],
                        muT_sb[:, di, q0:q0 + P],
                        start=(di == 0), stop=(di == 3))
            qh = qh_pool.tile([P, 4, P], f32, tag="qh")
            with tc.high_priority(offset=120):
                nc.scalar.copy(qh[:], qh_ps[:])

            # scores strip [128 q, W keys] fp32; per-chunk top-8 candidates
            # (overlaps the max scan with remaining chunks' matmuls; only
            # find_index8 stays serial after the full strip)
            strip = strip_pool.tile([P, N], f32, tag="strip")
            cand = top_pool.tile([P, 64], f32, tag="cand")
            nchunks = (W + 511) // 512
            for c in range(nchunks):
                k0 = c * 512
                csz = min(512, W - k0)
                sps = ps_score.tile([P, 512], f32, tag="sps")
                for di in range(4):
                    nc.tensor.matmul(
                        sps[:, :csz],
                        qh[:, di, :],
                        muT_sb[:, di, 4 + k0:4 + k0 + csz],
                        start=(di == 0), stop=(di == 3))
                nc.scalar.copy(strip[:, k0:k0 + csz], sps[:, :csz])
                # masks must land before this chunk's max scan
                if k0 <= 0 < k0 + csz:
                    nc.vector.tensor_tensor(
                        strip[:, 0:P], strip[:, 0:P], headmask_sb[:],
                        op=OP.add)
                if c == nchunks - 1:
                    nc.vector.tensor_tensor(
                        strip[:, W - P:W], strip[:, W - P:W], trimask_sb[:],
                        op=OP.add)
                nc.vector.max(out=cand[:, 8 * c:8 * c + 8],
                              in_=strip[:, k0:k0 + csz])

            # top-8 values + indices
            v8 = top_pool.tile([P, TOPK], f32, tag="v8")
            nc.vector.max(out=v8[:], in_=cand[:, :8 * nchunks])
            i8 = top_pool.tile([P, TOPK], dt.uint32, tag="i8")
            nc.vector.max_index(out=i8[:], in_max=v8[:], in_values=strip[:, :W])
            # softmax over the 8 (masked entries are ~-1e30 -> exp ~ 0)
            nmax = top_pool.tile([P, 1], f32, tag="nmax")
            nc.vector.tensor_scalar_mul(nmax[:], v8[:, 0:1], -1.0)
            e8 = top_pool.tile([P, TOPK], f32, tag="e8")
            zsum = top_pool.tile([P, 1], f32, tag="zsum")
            nc.scalar.activation(e8[:], v8[:], AF.Exp, bias=nmax[:],
                                 accum_out=zsum[:])
            zr = top_pool.tile([P, 1], f32, tag="zr")
            nc.vector.reciprocal(zr[:], zsum[:])
            p8 = top_pool.tile([P, TOPK], f32, tag="p8")
            nc.vector.tensor_scalar_mul(p8[:], e8[:], zr[:])

            # gather index layout roundtrip (wrapped int16 for dma_gather):
            # wrapped[(k*128+p)%16, (k*128+p)//16] = i8[p, k], built with a
            # DRAM roundtrip + 8 replicate loads.  Issue the DMAs here; the
            # DVE cast + gather issue go in emit_front_b so wsum(s-1) can
            # run on the DVE during the DMA flight time.
            nc.sync.dma_start(iw_dram[s], i8[:])
            iw32 = top_pool.tile([P, TOPK * P // 16], dt.uint32, tag="iw32")
            flat = iw_dram[s].rearrange("a b -> (a b)")
            for rep in range(8):
                src_ap = bass.AP(flat.tensor, flat.offset,
                                 [[8, 16], [1, TOPK], [TOPK * 16, 8]])
                nc.sync.dma_start(
                    iw32[16 * rep:16 * rep + 16, :]
                    .rearrange("p (k h) -> p k h", k=TOPK), src_ap)
            slot_state[s] = (p8, iw32, globalT)

        def emit_front_b(s):
            """iw cast + gather issue for slot s (after back_dve(s-1))."""
            p8, iw32, globalT = slot_state.pop(s)
            iw = top_pool.tile([P, TOPK * P // 16], dt.int16, tag="iw")
            nc.vector.tensor_copy(iw[:], iw32[:])
            g = g_pool.tile([P, TOPK, D], b16, tag="g")
            nc.gpsimd.dma_gather(g[:], mukeys[:], iw[:], num_idxs=TOPK * P,
                                 num_idxs_reg=TOPK * P, elem_size=D)
            slot_state[s] = (p8, g, globalT)

        acc_state = {}

        def emit_back_dve(s):
            """Weighted sum for slot s (lag 1, emitted BEFORE the next front
            so it sits ahead of the strip-gated DVE chain in the FIFO)."""
            p8, g, globalT = slot_state.pop(s)
            # gsum[q, :] = sum_k p8[q,k] * g[q,k,:] (smallest weights first)
            acc = acc_pool.tile([P, D], b16, tag="acc")
            nc.vector.tensor_scalar_mul(acc[:], g[:, TOPK - 1, :],
                                        p8[:, TOPK - 1:TOPK])
            for k in range(TOPK - 2, -1, -1):
                nc.vector.scalar_tensor_tensor(
                    acc[:], g[:, k, :], p8[:, k:k + 1], acc[:],
                    op0=OP.mult, op1=OP.add)
            acc_state[s] = (acc, globalT)

        def emit_back_pe(s):
            """Transpose gsum -> globalT for slot s (lag 4: acc is ready well
            before the PE reaches these, so the PE FIFO rarely waits)."""
            acc, globalT = acc_state.pop(s)
            for j in range(4):
                tp = ps_tp.tile([P, P], b16, tag="tp")
                nc.tensor.transpose(tp[:], acc[:, j * P:(j + 1) * P], ident16[:])
                nc.vector.tensor_copy(
                    globalT[:, j, (s % 4) * P:(s % 4 + 1) * P], tp[:])

        def emit_local_mm(grp, hbar):
            """Local branch for own-tile group grp (4 own tiles, 512 rows):
            hbar = sum_w silu(A + B_shift(w) + bm1), group-batched.
            muloc strips: per own tile, bf16 rows [128t-4, 128t+128) as 132
            cols (zero-padded at n<0)."""
            muloc_g = loc_pref.pop(grp)
            a_sb = loc_pool.tile([P, 4, D], b16, tag="a_sb")
            b_sb = loc_pool.tile([P, 4, 2, 264], b16, tag="b_sb")
            for dh in range(4):
                a_ps = ps_a.tile([P, D], f32, tag="a_ps")
                for di in range(4):
                    mv = muloc_g[:, di, :] \
                        .rearrange("p (t c) -> p t c", c=132)[:, :, 4:132]
                    nc.tensor.matmul(
                        a_ps[:].rearrange("p (t c) -> p t c", c=128),
                        wm1t_sb[:, di, dh * P:(dh + 1) * P],
                        mv, start=(di == 0), stop=(di == 3))
                nc.scalar.activation(a_sb[:, dh, :], a_ps[:], AF.Identity,
                                     bias=bm1t_sb[:, dh:dh + 1])
                for half in range(2):
                    b_ps = ps_b.tile([P, 2, 132], f32, tag=f"b_ps{half}")
                    for di in range(4):
                        mv = muloc_g[:, di, 264 * half:264 * half + 264] \
                            .rearrange("p (t c) -> p t c", c=132)
                        nc.tensor.matmul(
                            b_ps[:], wm1b_sb[:, di, dh * P:(dh + 1) * P],
                            mv, start=(di == 0), stop=(di == 3))
                    nc.scalar.copy(
                        b_sb[:, dh, half, :].rearrange("p (t c) -> p t c", c=132),
                        b_ps[:])
            loc_state[grp] = (a_sb, b_sb, hbar)

        def emit_local_w(grp, w):
            """One shift w of the local branch: x = A(+bm1) + B_shift(w),
            hbar (+)= silu(x).  Emitted interleaved with the next group's
            slots to spread ACT/DVE load."""
            a_sb, b_sb, hbar = loc_state[grp]
            x = locw_pool.tile([P, 4, D], b16, tag="x")
            for dh in range(4):
                in0 = a_sb[:, dh, :].rearrange(
                    "p (a t c) -> p a t c", a=2, c=128)
                in1 = b_sb[:, dh].rearrange(
                    "p a (t c) -> p a t c", c=132)[:, :, :, 4 - w:132 - w]
                outw = x[:, dh, :].rearrange(
                    "p (a t c) -> p a t c", a=2, c=128)
                nc.vector.tensor_tensor(outw, in0, in1, op=OP.add)
            if w == 1:
                nc.scalar.activation(
                    hbar[:].rearrange("p a c -> p (a c)"),
                    x[:].rearrange("p a c -> p (a c)"), AF.Silu)
            else:
                sil = locw_pool.tile([P, 4, D], b16, tag="sil")
                nc.scalar.activation(
                    sil[:].rearrange("p a c -> p (a c)"),
                    x[:].rearrange("p a c -> p (a c)"), AF.Silu)
                nc.vector.tensor_tensor(
                    hbar[:].rearrange("p a c -> p (a c)"),
                    hbar[:].rearrange("p a c -> p (a c)"),
                    sil[:].rearrange("p a c -> p (a c)"), op=OP.add)

        def emit_outproj(grp):
            _, _, hbar = loc_state[grp]
            globalT = gt_done.pop(grp)
            r0 = grp * 512
            for do in range(4):
                o_ps = ps_o.tile([P, 512], f32, tag="o_ps")
                for dm in range(4):
                    nc.tensor.matmul(
                        o_ps[:],
                        wmo_sb[:, dm, do * P:(do + 1) * P],
                        hbar[:, dm, :],
                        start=(dm == 0), stop=False)
                for dm in range(4):
                    nc.tensor.matmul(
                        o_ps[:],
                        wvo_sb[:, dm, do * P:(do + 1) * P],
                        globalT[:, dm, :],
                        start=False, stop=(dm == 3))
                ost = out_pool.tile([P, 512], f32, tag="ost")
                nc.scalar.activation(ost[:], o_ps[:], AF.Identity,
                                     bias=bconst_sb[:, do:do + 1])
                nc.sync.dma_start(outT[do, :, r0:r0 + 512], ost[:])

        gt_done = {}
        loc_pref = {}

        def emit_local_pref(grp):
            muloc_g = muloc_pool.tile([P, 4, 528], b16, tag="mulocg")
            for di in range(4):
                nc.sync.dma_start(muloc_g[:, di, :],
                                  muloc[di][:, 528 * grp:528 * grp + 528])
            loc_pref[grp] = muloc_g
        # Software-pipelined emission: slot fronts run one ahead of backs;
        # local silu work and outproj lag a full group so nothing on the
        # PE/ACT FIFOs ever waits on a gather chain.
        for grp in range(4):
            globalT = gt_pool.tile([P, 4, 512], b16, tag="globalT")
            gt_done[grp] = globalT
            hbar = hbar_pool.tile([P, 4, 512], b16, tag="hbar")
            emit_local_pref(grp)
            for j, s in enumerate(range(4 * grp, 4 * grp + 4)):
                emit_front(s, globalT)
                if s > 1:
                    with tc.high_priority(offset=400):
                        emit_back_dve(s - 2)
                emit_front_b(s)
                if s > 3:
                    with tc.high_priority(offset=150):
                        emit_back_pe(s - 4)
                if grp > 0:
                    for w in ([1, 2] if j == 0 else [3, 4] if j == 1 else []):
                        emit_local_w(grp - 1, w)
            if grp > 0:
                emit_outproj(grp - 1)
            emit_local_mm(grp, hbar)
        emit_back_dve(NSLOT - 2)
        emit_back_dve(NSLOT - 1)
        for s in range(NSLOT - 4, NSLOT):
            emit_back_pe(s)
        for w in range(1, WIN + 1):
            emit_local_w(3, w)
        emit_outproj(3)

    nc.compile()
    _cache["nc"] = nc
    return nc


def prep_in_maps(inputs):
    f32 = np.float32
    b16 = ml_dtypes.bfloat16
    mu = np.asarray(inputs["mu"], f32)
    Wq = np.asarray(inputs["Wq"], f32)
    bq = np.asarray(inputs["bq"], f32)
    Wk = np.asarray(inputs["Wk"], f32)
    Wv = np.asarray(inputs["Wv"], f32)
    bv = np.asarray(inputs["bv"], f32)
    Wm1 = np.asarray(inputs["Wm1"], f32)
    bm1 = np.asarray(inputs["bm1"], f32)
    Wm2 = np.asarray(inputs["Wm2"], f32)
    bm2 = np.asarray(inputs["bm2"], f32)
    Wo = np.asarray(inputs["Wo"], f32)
    bo = np.asarray(inputs["bo"], f32)
    assert not bq.any(), "bq != 0 unsupported (adds a per-key score term)"

    Wqks = (Wq @ Wk.T / math.sqrt(D)).astype(f32)
    Wmo = ((Wm2 @ Wo[:D]) / WIN).astype(f32)
    Wvo = (Wv @ Wo[D:]).astype(f32)
    bconst = (bo + bm2 @ Wo[:D] + bv @ Wo[D:]).astype(f32)
    consts = dict(
        wqks=np.ascontiguousarray(Wqks.reshape(4, P, D)),
        wm1t=np.ascontiguousarray(Wm1[:D]).reshape(4, P, D).astype(b16),
        wm1b=np.ascontiguousarray(Wm1[D:]).reshape(4, P, D).astype(b16),
        wmo=Wmo.reshape(4, P, D).astype(b16),
        wvo=Wvo.reshape(4, P, D).astype(b16),
        bm1t=np.ascontiguousarray(bm1.reshape(4, P).T),
        bconst=np.ascontiguousarray(bconst.reshape(4, P).T),
    )

    j = np.arange(P)[None, :]
    p = np.arange(P)[:, None]
    tril0 = np.where(j <= p, 0.0, NEG).astype(f32)

    in_maps = []
    for c in range(NCORES):
        b, h = c // 2, c % 2
        mub = np.ascontiguousarray(mu[b])                   # [N, D] f32
        muT_pad = np.zeros((4, P, NPAD), f32)
        keys = np.zeros((N, D), f32)
        if h == 1:
            muT_pad[:, :, 4:] = mub.T.reshape(4, P, N)
            keys[:] = mub
            hm = np.zeros((P, P), f32)
        else:
            # shift mu right by 128 key-columns: view[n'] = mu[n'-128]
            muT_pad[:, :, 4 + P:] = mub[:N - P].T.reshape(4, P, N - P)
            keys[P:] = mub[:N - P]
            hm = np.full((P, P), NEG, f32)
        # local strips: per own tile t, rows [128t-4, 128t+128) zero-padded
        strips = []
        for t in range(h, 32, 2):
            st = np.zeros((132, D), f32)
            lo = 128 * t - 4
            src_lo = max(lo, 0)
            st[src_lo - lo:] = mub[src_lo:128 * t + 128]
            strips.append(st)
        muloc = np.concatenate(strips)                      # [2112, D]
        muloc = np.ascontiguousarray(muloc.T).reshape(4, P, NSLOT * 132)
        in_maps.append(dict(
            muT=muT_pad,
            muloc=muloc.astype(b16),
            mukeys=keys.astype(b16),
            trimask=tril0,
            headmask=hm,
            **consts,
        ))
    return in_maps


def assemble(core_outs):
    """core_outs: list of outT arrays [4, P, 2048] per core -> full [B, N, D]."""
    out = np.empty((B, N, D), np.float32)
    for c in range(NCORES):
        b, h = c // 2, c % 2
        oT = np.asarray(core_outs[c])
        oc = np.ascontiguousarray(oT.reshape(D, NSLOT * P).T)  # [2048, D]
        for s, t in enumerate(range(h, 32, 2)):
            out[b, 128 * t:128 * t + 128] = oc[128 * s:128 * s + 128]
    return out


def kernel(**inputs):
    nc = _build_program()
    in_maps = prep_in_maps(inputs)

    import os
    from concourse.bass_utils import run_bass_kernel_spmd
    trace = bool(int(os.environ.get("LR_TRACE", "0")))
    res = run_bass_kernel_spmd(nc, in_maps, core_ids=list(range(NCORES)),
                               trace=trace)
    _cache["last_results"] = res
    return assemble([res.results[c]["outT"] for c in range(NCORES)])
